# revision 3
# baseline (speedup 1.0000x reference)
"""Trainium2 Bass kernel for causal masked-ReLU attention (no softmax).

Reference computation (B=8, T=1024, C=768, n_head=12, hd=64):
    qkv = x @ W_attn.T + b_attn
    q, k, v = split(qkv); per-head: att = relu(mask_causal(q k^T / sqrt(hd)))
    y = att @ v, heads re-merged -> (B, T, C)

Sharding: one batch element per NeuronCore (8 cores). Each core computes the
QKV projection and all 12 heads' attention for its batch element.

Layout strategy (per core):
  - Host passes x[b].T (C, T) and W.T (C, 3C) so the contraction dim C lands
    on SBUF partitions with unit-stride DMA (no on-chip transposes).
  - W rows are pre-permuted on host into [q-pair0, k-pair0, q-pair1, ...] so
    q.T / k.T of head h live at the same partition offset (h%2)*64 of their
    M-tiles; matmul operands then share a base partition.
  - q weights/bias are pre-scaled by 1/sqrt(hd) on host.
  - QKV projection runs in fp8 (e4m3) DoubleRow perf mode: 256-deep
    contraction per pass at 0.5 cycles/row. Operands are split into hi+lo
    fp8 digits (x = xh + xl, W = wh + wl, pre-scaled into e4m3's normal
    range) and three digit products xh*wh + xl*wh + xh*wl accumulate in one
    fp32 PSUM group; the dropped xl*wl term is ~1e-4 relative. The 2^13
    operand scaling is removed at eviction, where the bias is also added.
    PE cost 0.75x of an fp16 projection.
  - att is computed transposed (att.T = k @ q.T, layout [T_k, T_q]), fp16
    everywhere, with BOTH heads of a pair sharing one [128, 2, 512]
    double-bank PSUM tile and one [128, 2, T] SBUF tile per k-tile: the
    per-instruction PSUM-access penalty on DVE/ACT (~125/185ns) is the
    phase-2 bottleneck, and pair-merged pieces halve the eviction op count.
  - The AV product exploits weight-stationary asymmetry: per (q-tile,
    k-tile, head) matmul the STATIONARY operand is the [128, 128] att.T
    block and the MOVING operand is the head's 64 v columns, so the PE
    streams only 64 columns per k-tile. Both heads accumulate into one
    [128, 512] PSUM bank (cols = 128*(t%4) + 64*head_parity + d); y evicts
    as [128, 512] copies and DMAs out with a 3-d strided AP into natural
    (T, C) layout.
  - Causal structure at 128-col granularity: fully-masked regions are never
    computed; diagonal blocks flow through a [tri(128) | ones] relu-mask at
    eviction. DVE owns every mask-needing piece (it alone has
    scalar_tensor_tensor among PSUM-capable engines; Pool cannot read
    PSUM); ACT owns the pure-relu [512:T) halves, tile 4's relu remainder,
    and most y copies. The split converges both engines at ~6.0us/pair,
    just above the PE's 5.8us/pair, the phase-2 critical path.
  - fp8 operands are packed partition-major on the host; the front of the
    input stream is split fine (x pair halves) and spread across BOTH DMA
    queues (SP HWDGE + Pool SWDGE, which bypasses the shared HWDGE's fixed
    ~625ns/DMA cost) because the first window's eviction needs all three
    contraction pairs on chip: single-queue it starves the PE ~3us.
  - A burst of warmup matmuls on a zeroed scratch tile runs during the
    initial DMA wait, and a few more are sprinkled between the first
    windows' term blocks: the PE p-state ramp needs ~3us of continuous
    busy to reach full clock, and idle gaps reset it.
  - One pool scope spans both phases (a pool close = all-engine barrier).
    Pair 0's first two k-tiles run as unmerged single-bank pieces through
    the spare ps_y ring so phase 2 starts while the last projection
    windows drain.
  - The final pair's bank-high eviction is split [t4,t5 | t6,t7] so the
    closing chain after the last matmul is one [128, 256] copy + small DMA.
  - Output is written as y (T, C) in fp16; host upcasts.
"""

import numpy as np

import sys
for _p in ("/opt/trn_rl_repo", "/root/.axon_site", "/root/.axon_site/_ro/trn_rl_repo",
           "/root/.axon_site/_ro/pypackages"):
    if _p not in sys.path:
        sys.path.append(_p)

import ml_dtypes

import concourse.bacc as bacc
import concourse.mybir as mybir
from concourse.alu_op_type import AluOpType
from concourse.tile import TileContext
from concourse.bass_utils import run_bass_kernel_spmd

B, T, C = 8, 1024, 768
NH, HD = 12, 64
C3 = 3 * C            # 2304
KT = C // 128         # 6  contraction tiles of the projection
NP = KT // 2          # 3  contraction pairs (DoubleRow)
TT = T // 128         # 8  tiles of the sequence dim
NPAIR = NH // 2       # 6  head pairs
F32 = mybir.dt.float32
F16 = mybir.dt.float16
F8 = mybir.dt.float8e4
AF = mybir.ActivationFunctionType
DR = mybir.MatmulPerfMode.DoubleRow

SX = 16.0             # x pre-scale (keeps x-lo digits in e4m3 normal range)
SW = 512.0            # W pre-scale
DESCALE = 1.0 / (SX * SW)

# warmup matmul moving-widths (fp16): ramp the PE p-state during the
# initial input-DMA wait so real matmuls start at full clock
WARM = [512] * 5 + [256] * 2
NSPRINKLE = 6         # ramp-guard warmups per early window term-block

_CACHE = {}


def _build():
    nc = bacc.Bacc("TRN2", target_bir_lowering=False, debug=False, num_devices=8)

    xh = nc.dram_tensor("xh", [128, KT, T], F8, kind="ExternalInput").ap()
    xl = nc.dram_tensor("xl", [128, KT, T], F8, kind="ExternalInput").ap()
    wvh = nc.dram_tensor("wvh", [128, KT, C], F8, kind="ExternalInput").ap()
    wvl = nc.dram_tensor("wvl", [128, KT, C], F8, kind="ExternalInput").ap()
    wqh = nc.dram_tensor("wqh", [128, KT, 2 * C], F8, kind="ExternalInput").ap()
    wql = nc.dram_tensor("wql", [128, KT, 2 * C], F8, kind="ExternalInput").ap()
    bqk = nc.dram_tensor("bqk", [128, 2 * NPAIR], F32, kind="ExternalInput").ap()
    bvb = nc.dram_tensor("bvb", [128, C], F16, kind="ExternalInput").ap()
    # masks = [tri(128) | ones(896)] duplicated along dim1 so pair-merged
    # [128, 2, W] evictions read the same relu-mask for both heads
    masks = nc.dram_tensor("masks", [128, 2, T], F32, kind="ExternalInput").ap()
    # y in natural (T, C) layout, tiled (TT, 128, C) for the 3-d AV DMAs
    y_d = nc.dram_tensor("y", [TT, 128, C], F16, kind="ExternalOutput").ap()

    with TileContext(nc) as tc:
        with (
            tc.tile_pool(name="persist", bufs=1) as pp,
        ):
            masks_sb = pp.tile([128, 2, T], F32, name="masks_sb")
            bqk_sb = pp.tile([128, 2 * NPAIR], F32, name="bqk_sb")
            bvb_sb = pp.tile([128, C], F16, name="bvb_sb")
            qkT = [pp.tile([128, T], F16, name=f"qkT{m}") for m in range(2 * NPAIR)]
            v_sb = [pp.tile([128, C], F16, name=f"v{t}") for t in range(TT)]
            # att.T tiles, fp16, dim1 = head parity (pair-merged)
            att2 = [pp.tile([128, 2, T], F16, name=f"att{t}") for t in range(TT)]

            # ---------- Phase 1: QKV projection (fp8 DoubleRow, 3 digit
            # products xh*wh + xl*wh + xh*wl into one PSUM group) ----------
            from contextlib import ExitStack
            with ExitStack() as stack:
                iop = stack.enter_context(tc.tile_pool(name="io", bufs=1))
                # 3 double-bank tiles (12KB/partition) shared by projection
                # windows and phase-2 merged QK pieces; + 2 single banks for
                # warmup / early-QK / AV
                ps_proj = stack.enter_context(
                    tc.tile_pool(name="psum_proj", bufs=3, space="PSUM"))
                ps_y = stack.enter_context(
                    tc.tile_pool(name="psum_y", bufs=2, space="PSUM"))
                yop = stack.enter_context(tc.tile_pool(name="yout", bufs=2))
                xh_sb = iop.tile([128, KT, T], F8, name="xh_sb")
                xl_sb = iop.tile([128, KT, T], F8, name="xl_sb")
                wv_h = iop.tile([128, KT, C], F8, name="wv_h")
                wv_l = iop.tile([128, KT, C], F8, name="wv_l")
                wq_h = iop.tile([128, KT, 2 * C], F8, name="wq_h")
                wq_l = iop.tile([128, KT, 2 * C], F8, name="wq_l")

                # PE p-state warmup on a zeroed scratch tile; results are
                # never read
                scratch = iop.tile([128, 512], F16, name="warm_src")
                nc.vector.memset(scratch[:], 0.0)
                warm = ps_y.tile([128, 512], F32, name="ps_warm", tag="ps_y")
                for w in WARM:
                    nc.tensor.matmul(warm[:, :w], scratch[:, :128],
                                     scratch[:, :w], start=True, stop=True)

                # input DMAs. The first window's eviction gates on ALL three
                # contraction pairs, so the front ships x in column halves
                # and spreads the lo-digit stream onto the Pool SWDGE queue
                # (its descriptor generation bypasses the shared HWDGE).
                nc.sync.dma_start(out=wv_h[:, 0:2, :], in_=wvh[:, 0:2, :])
                nc.sync.dma_start(out=xh_sb[:, 0:2, 0:256], in_=xh[:, 0:2, 0:256])
                nc.gpsimd.dma_start(out=xl_sb[:, 0:2, 0:512], in_=xl[:, 0:2, 0:512])
                nc.sync.dma_start(out=xh_sb[:, 0:2, 256:512], in_=xh[:, 0:2, 256:512])
                nc.gpsimd.dma_start(out=wv_l[:, 0:2, :], in_=wvl[:, 0:2, :])
                for p in range(1, NP):
                    pr = slice(2 * p, 2 * p + 2)
                    nc.sync.dma_start(out=xh_sb[:, pr, 0:512], in_=xh[:, pr, 0:512])
                    nc.gpsimd.dma_start(out=xl_sb[:, pr, 0:512], in_=xl[:, pr, 0:512])
                    nc.sync.dma_start(out=wv_h[:, pr, :], in_=wvh[:, pr, :])
                    nc.gpsimd.dma_start(out=wv_l[:, pr, :], in_=wvl[:, pr, :])
                nc.sync.dma_start(out=bvb_sb[:], in_=bvb[:])
                # x column rests (tiles 4-7 of the seq dim)
                for p in range(NP):
                    pr = slice(2 * p, 2 * p + 2)
                    nc.sync.dma_start(out=xh_sb[:, pr, 512:T], in_=xh[:, pr, 512:T])
                    nc.gpsimd.dma_start(out=xl_sb[:, pr, 512:T], in_=xl[:, pr, 512:T])
                nc.sync.dma_start(out=bqk_sb[:], in_=bqk[:])
                # pair 0's q/k weights ship in m0-m3 / m4-m11 halves so the
                # first qk windows start earlier
                pr0 = slice(0, 2)
                nc.sync.dma_start(out=wq_h[:, pr0, :512], in_=wqh[:, pr0, :512])
                nc.gpsimd.dma_start(out=wq_l[:, pr0, :512], in_=wql[:, pr0, :512])
                nc.sync.dma_start(out=wq_h[:, pr0, 512:], in_=wqh[:, pr0, 512:])
                nc.gpsimd.dma_start(out=wq_l[:, pr0, 512:], in_=wql[:, pr0, 512:])
                for p in range(1, NP):
                    prp = slice(2 * p, 2 * p + 2)
                    nc.sync.dma_start(out=wq_h[:, prp, :], in_=wqh[:, prp, :])
                    nc.gpsimd.dma_start(out=wq_l[:, prp, :], in_=wql[:, prp, :])
                nc.sync.dma_start(out=masks_sb[:], in_=masks[:])

                # each group = one [128, 512] PSUM bank (one [:, j, :] lane
                # of a double-bank tile) holding one or two 256-wide
                # DoubleRow chunks. ("v", t, n0, width) / ("qk", m, q0, width)
                groups = []
                for t in range(TT):
                    groups.append(("v", t, 0, 512))
                    groups.append(("v", t, 512, 256))
                for m in range(2 * NPAIR):
                    for q0 in (0, 512):
                        groups.append(("qk", m, q0, 512))

                # windows of 4 groups = 2 double-bank tiles; k-pair-major,
                # digit-product-minor within the window so the PE's
                # consumption order matches the DMA arrival order.
                for wi, w0 in enumerate(range(0, len(groups), 4)):
                    window = groups[w0:w0 + 4]
                    dbl = [ps_proj.tile([128, 2, 512], F32, name="ps_proj",
                                        tag="ps_proj")
                           for _ in range((len(window) + 1) // 2)]
                    tiles = [dbl[gi // 2][:, gi % 2, :]
                             for gi in range(len(window))]
                    nmm = [0] * len(window)
                    total = [9 * (g[3] // 256) for g in window]
                    for p in range(NP):
                        pr = slice(2 * p, 2 * p + 2)
                        for term in range(3):
                            xa = (xh_sb, xl_sb, xh_sb)[term]
                            wva = (wv_h, wv_h, wv_l)[term]
                            wqa = (wq_h, wq_h, wq_l)[term]
                            for gi, (g, ps) in enumerate(zip(window, tiles)):
                                kind, i, o0, wd = g
                                for c0 in range(0, wd, 256):
                                    n = nmm[gi]
                                    nmm[gi] = n + 1
                                    st = n == 0
                                    sp = n == total[gi] - 1
                                    if kind == "v":
                                        nc.tensor.matmul(
                                            ps[:, c0:c0 + 256],
                                            xa[:, pr, 128 * i:128 * (i + 1)],
                                            wva[:, pr, o0 + c0:o0 + c0 + 256],
                                            start=st, stop=sp, perf_mode=DR,
                                        )
                                    else:
                                        nc.tensor.matmul(
                                            ps[:, c0:c0 + 256],
                                            wqa[:, pr, 128 * i:128 * (i + 1)],
                                            xa[:, pr, o0 + c0:o0 + c0 + 256],
                                            start=st, stop=sp, perf_mode=DR,
                                        )
                            if wi < 2 and p == 1 and term == 2:
                                # ramp-guard warmups: fill the stall while
                                # pair 2 is still in flight so the PE
                                # p-state doesn't reset
                                for _ in range(NSPRINKLE):
                                    nc.tensor.matmul(
                                        warm[:, :128], scratch[:, :128],
                                        scratch[:, :128], start=True,
                                        stop=True)
                    for g, ps in zip(window, tiles):
                        kind, i, o0, wd = g
                        if kind == "v":
                            nc.vector.scalar_tensor_tensor(
                                v_sb[i][:, o0:o0 + wd], ps[:, :wd], DESCALE,
                                bvb_sb[:, o0:o0 + wd],
                                AluOpType.mult, AluOpType.add,
                            )
                        elif i % 2 == 0:
                            nc.scalar.activation(
                                qkT[i][:, o0:o0 + wd], ps[:, :wd],
                                AF.Identity, bias=bqk_sb[:, i:i + 1],
                                scale=DESCALE,
                            )
                        else:
                            nc.vector.tensor_scalar(
                                qkT[i][:, o0:o0 + wd], ps[:, :wd],
                                DESCALE, bqk_sb[:, i:i + 1],
                                AluOpType.mult, AluOpType.add,
                            )

            # ---------- Phase 2: attention, pair by pair (same pool
            # scope: no phase barrier) ----------
                for a in range(NPAIR):
                    qa, ka = qkT[2 * a], qkT[2 * a + 1]
                    # ---- QK^T -> att.T, both heads into one double-bank
                    # piece, tk ascending. DVE takes every mask-needing
                    # piece, ACT the pure-relu halves; tk4 splits
                    # [masked 2x128 | relu 2x384] to balance the two. ----
                    for tk in range(TT):
                        k0 = 128 * tk
                        q0d = 256 * (tk // 2)   # start of diag window
                        if q0d < 512:
                            if a == 0 and tk <= 1:
                                # unmerged singles through the spare ps_y
                                # ring: starts while the last projection
                                # windows still hold ps_proj
                                for r in range(2):
                                    kh = ka[64 * r:64 * (r + 1), :]
                                    qh = qa[64 * r:64 * (r + 1), :]
                                    ps = ps_y.tile([128, 512], F32,
                                                   name="ps_qk", tag="ps_y")
                                    nc.tensor.matmul(
                                        ps[:, k0 - q0d:512 - q0d],
                                        kh[:, k0:k0 + 128], qh[:, k0:512],
                                        start=True, stop=True,
                                    )
                                    nc.vector.scalar_tensor_tensor(
                                        att2[tk][:, r, k0:512],
                                        ps[:, k0 - q0d:512 - q0d],
                                        0.0, masks_sb[:, 0, :512 - k0],
                                        AluOpType.max, AluOpType.mult,
                                    )
                                    ps = ps_y.tile([128, 512], F32,
                                                   name="ps_qk", tag="ps_y")
                                    nc.tensor.matmul(
                                        ps[:], kh[:, k0:k0 + 128],
                                        qh[:, 512:T],
                                        start=True, stop=True,
                                    )
                                    nc.scalar.activation(
                                        att2[tk][:, r, 512:T], ps[:],
                                        AF.Relu)
                                continue
                            # piece 1: [k0, 512), masked relu on DVE
                            ps = ps_proj.tile([128, 2, 512], F32,
                                              name="ps_qk", tag="ps_proj")
                            for r in range(2):
                                nc.tensor.matmul(
                                    ps[:, r, k0 - q0d:512 - q0d],
                                    ka[64 * r:64 * (r + 1), k0:k0 + 128],
                                    qa[64 * r:64 * (r + 1), k0:512],
                                    start=True, stop=True,
                                )
                            nc.vector.scalar_tensor_tensor(
                                att2[tk][:, :, k0:512],
                                ps[:, :, k0 - q0d:512 - q0d],
                                0.0, masks_sb[:, :, :512 - k0],
                                AluOpType.max, AluOpType.mult,
                            )
                            # piece 2: the full unmasked [512, 1024) half
                            ps = ps_proj.tile([128, 2, 512], F32,
                                              name="ps_qk", tag="ps_proj")
                            for r in range(2):
                                nc.tensor.matmul(
                                    ps[:, r, :],
                                    ka[64 * r:64 * (r + 1), k0:k0 + 128],
                                    qa[64 * r:64 * (r + 1), 512:T],
                                    start=True, stop=True,
                                )
                            nc.scalar.activation(att2[tk][:, :, 512:T],
                                                 ps[:, :, :], AF.Relu)
                        else:
                            # single piece [k0, 1024)
                            ps = ps_proj.tile([128, 2, 512], F32,
                                              name="ps_qk", tag="ps_proj")
                            pw = T - q0d
                            for r in range(2):
                                nc.tensor.matmul(
                                    ps[:, r, k0 - q0d:pw],
                                    ka[64 * r:64 * (r + 1), k0:k0 + 128],
                                    qa[64 * r:64 * (r + 1), k0:T],
                                    start=True, stop=True,
                                )
                            if tk == 4:
                                nc.vector.scalar_tensor_tensor(
                                    att2[tk][:, :, k0:k0 + 128],
                                    ps[:, :, 0:128],
                                    0.0, masks_sb[:, :, :128],
                                    AluOpType.max, AluOpType.mult,
                                )
                                nc.scalar.activation(
                                    att2[tk][:, :, k0 + 128:T],
                                    ps[:, :, 128:pw], AF.Relu)
                            else:
                                nc.vector.scalar_tensor_tensor(
                                    att2[tk][:, :, k0:T],
                                    ps[:, :, k0 - q0d:pw],
                                    0.0, masks_sb[:, :, :T - k0],
                                    AluOpType.max, AluOpType.mult,
                                )

                    # ---- AV: y[q, d] per q-tile, att.T block stationary,
                    # v columns moving (64 per k-tile); both heads pack one
                    # [128, 512] bank: cols = 128*(t%4) + 64*r + d ----
                    for bk in range(2):
                        ps2 = ps_y.tile([128, 512], F32, name="ps_av",
                                        tag="ps_y")
                        y_sb = yop.tile([128, 512], F16, name="y_sb",
                                        tag="y_sb")
                        for t in range(4 * bk, 4 * bk + 4):
                            for r in range(2):
                                h = 2 * a + r
                                col = 128 * (t % 4) + 64 * r
                                for k in range(t + 1):
                                    nc.tensor.matmul(
                                        ps2[:, col:col + 64],
                                        att2[k][:, r, 128 * t:128 * (t + 1)],
                                        v_sb[k][:, 64 * h:64 * (h + 1)],
                                        start=(k == 0), stop=(k == t),
                                    )
                        if bk == 0:
                            nc.scalar.copy(y_sb[:], ps2[:])
                            nc.sync.dma_start(
                                out=y_d[0:4, :, 128 * a:128 * (a + 1)]
                                    .transpose([1, 0, 2]),
                                in_=y_sb[:])
                        else:
                            # split eviction: [t4,t5] on DVE as soon as
                            # their groups stop, [t6,t7] on ACT; keeps the
                            # closing chain after the last matmul small
                            nc.vector.tensor_scalar(
                                y_sb[:, 0:256], ps2[:, 0:256], 0.0, None,
                                AluOpType.add)
                            nc.sync.dma_start(
                                out=y_d[4:6, :, 128 * a:128 * (a + 1)]
                                    .transpose([1, 0, 2]),
                                in_=y_sb[:, 0:256])
                            nc.scalar.copy(y_sb[:, 256:512], ps2[:, 256:512])
                            nc.sync.dma_start(
                                out=y_d[6:8, :, 128 * a:128 * (a + 1)]
                                    .transpose([1, 0, 2]),
                                in_=y_sb[:, 256:512])

    nc.compile()
    return nc


def _prep_host(x, W_attn, b_attn):
    s = 1.0 / np.sqrt(np.float32(HD))
    W = np.asarray(W_attn, dtype=np.float32).copy()
    b = np.asarray(b_attn, dtype=np.float32).copy()
    W[:C] *= s
    b[:C] *= s
    # interleave q/k head pairs: [q-pair0, k-pair0, q-pair1, k-pair1, ...], v natural
    rows = []
    for a in range(NPAIR):
        rows.extend(range(128 * a, 128 * (a + 1)))          # q heads 2a, 2a+1
        rows.extend(range(C + 128 * a, C + 128 * (a + 1)))  # k heads 2a, 2a+1
    rows.extend(range(2 * C, 3 * C))                        # v natural
    W_perm = W[rows]
    b_perm = b[rows]

    e4 = ml_dtypes.float8_e4m3

    def pack(mat):
        # (C, N) -> partition-major (128, KT, N): each partition's six
        # contraction k-tiles contiguous, k-pair-major
        Cr, N = mat.shape
        return np.ascontiguousarray(
            mat.reshape(KT, 128, N).transpose(1, 0, 2))

    def split8(mat):
        hi = mat.astype(e4)
        lo = (mat - hi.astype(np.float32)).astype(e4)
        return hi, lo

    wT = np.ascontiguousarray(W_perm.T) * np.float32(SW)     # (C, 3C)
    wqh, wql = split8(pack(wT[:, :2 * C]))
    wvh, wvl = split8(pack(wT[:, 2 * C:]))
    bqk = np.ascontiguousarray(b_perm[:2 * C].reshape(2 * NPAIR, 128).T)  # (128, 12)
    bvb = np.ascontiguousarray(
        np.broadcast_to(b_perm[2 * C:], (128, C))).astype(np.float16)
    tri = (np.arange(128)[None, :] >= np.arange(128)[:, None]).astype(np.float32)
    m1 = np.ones((128, T), dtype=np.float32)
    m1[:, 0:128] = tri             # kept windows always start at the diagonal
    masks = np.ascontiguousarray(
        np.broadcast_to(m1[:, None, :], (128, 2, T)))
    xT = np.asarray(x, dtype=np.float32).transpose(0, 2, 1) * np.float32(SX)  # (B, C, T)
    xhv = np.stack([pack(xT[c]) for c in range(B)])
    xhv, xlv = split8(xhv)
    return xhv, xlv, wqh, wql, wvh, wvl, bqk, bvb, masks


def kernel(x, W_attn, b_attn):
    if "nc" not in _CACHE:
        _CACHE["nc"] = _build()
    nc = _CACHE["nc"]

    (xhv, xlv, wqh, wql, wvh, wvl, bqk, bvb, masks) = _prep_host(x, W_attn, b_attn)
    in_maps = [
        {"xh": xhv[c], "xl": xlv[c], "wqh": wqh, "wql": wql, "wvh": wvh,
         "wvl": wvl, "bqk": bqk, "bvb": bvb, "masks": masks}
        for c in range(B)
    ]
    res = run_bass_kernel_spmd(nc, in_maps, list(range(B)))
    y = np.empty((B, T, C), dtype=np.float32)
    for c in range(B):
        y[c] = res.results[c]["y"].reshape(T, C).astype(np.float32)
    return y
